# revision 1
# baseline (speedup 1.0000x reference)
"""GCN block (GraphConv + BatchNorm1d + ReLU) on 8 Trainium2 NeuronCores.

Strategy: every core computes h = x @ W for ALL nodes (replicated matmul) so
no AllGather of node features is needed -- the extra x reads (51 MB vs 6.4 MB
per core) are far cheaper than a 25 MB collective. Nodes are partitioned by
dst across cores for the aggregation; W/b/gamma/beta are replicated and only
the 1 KB BN batch statistics cross cores (AllGather + on-device reduce).

Host-side preprocessing (integer index bookkeeping only):
  * dst side: nodes are serpentine-dealt (by in-degree, descending) into the
    C*NG (core, 128-row group) bins, equalizing per-bin in-degree totals;
    empty bin slots are masked out of the BN stats by a per-slot mask tile,
    and the output rows are unpermuted on the host.
  * src side: nodes with outgoing edges are greedily assigned to int16-sized
    banks balancing every (core, bank, group) bucket count. Together the two
    permutations shrink the shared (SPMD max-over-cores) gather padding to
    ~1% of the edge count.

Device pipeline, per core k:
  B. h = x @ W for all N nodes (bf16) written to per-bank HBM tables; the
     SBUF->HBM write uses a paired-row layout (partition p holds table rows
     2p/2p+1 of each 256-row group) so DMA descriptors are 512 B, not 256 B.
  D. Per (4-group chunk, bank): dma_gather the chunk's edges' h[src] rows
     (bf16, ~30 blocks of 128 edges per call; a few early-bank gathers are
     hoisted ahead in the Pool FIFO to fill stage-B DMA stalls) and
     segment-sum them with one-hot matmuls M^T @ G. Each group's chain
     accumulates across all banks in its own PSUM bank (accumulation groups
     are PSUM-bank-scoped). M columns are built per 128-edge block by one
     DVE tensor_scalar (iota == dst_offset) * rsqrt(deg_out[src]) -- the
     two-scalar form hits the DVE 4x perf mode and folds the source-side
     norm in for free. A block straddling two buckets runs two matmuls
     (offsets relative to each bucket; non-members hold 255 -> zero column).
  E. At chain stop: relu(psum * rsqrt(clip(deg_in,1)) + bias) -> agg (bf16);
     masked BN partial sums accumulate on two PSUM chains.
  F. AllGather of the 1 KB stats + partition_all_reduce; build affine S/T.
  G. y = relu_h * S + T in place (bf16, DVE 2x), chunked DMA out.

All floating-point math runs on device; the host only does integer
bucketing/permutations, degree counting (bincount), and dtype casts.
"""
import sys

sys.path.insert(0, "/opt/trn_rl_repo")

import numpy as np

import concourse.bacc as bacc
import concourse.bass as bass
import concourse.bass_isa as bass_isa
import concourse.mybir as mybir
import concourse.tile as tile
from concourse import bass_utils

F32 = mybir.dt.float32
BF16 = mybir.dt.bfloat16
I16 = mybir.dt.int16

CFG = dict(
    N=100000,
    E=1600000,
    IN=256,
    OUT=128,
    NCORES=8,
    GRP=128,          # dst nodes per segment group (= psum partition dim)
    BANKCAP=32512,    # max rows per src bank (int16 gather-index limit)
    GCHUNK=4,         # dst groups per chunk (concurrent PSUM accum chains;
                      # each chain needs its own PSUM bank)
    CH=4,            # x@W chunk size in 256-node pair-groups
    EPS=1e-5,
    TRACE=False,
)

LAST_RESULTS = None  # set by kernel() for test harness introspection
LAST_NC = None
LAST_RUN_S = None


def _ceil_div(a, b):
    return (a + b - 1) // b


def _wrap16(idx, ncols):
    """int16 idx list -> [128, ncols] tile: idx i at [i%16, i//16], replicated
    8x across the 16-partition groups (one copy per GpSimd Q7 core)."""
    n = idx.shape[0]
    assert n == ncols * 16
    w = np.ascontiguousarray(idx.reshape(ncols, 16).T)
    return np.tile(w, (8, 1))


def _preprocess(cfg, src, dst):
    """Bucket edges by (owner core, src bank, dst group); build per-core
    gather-index / dst-offset arrays and the shared block structure."""
    N, E = cfg["N"], cfg["E"]
    C, GRP = cfg["NCORES"], cfg["GRP"]
    NPC = N // C
    NG = _ceil_div(NPC, GRP)

    src = src.astype(np.int64)
    dst = dst.astype(np.int64)
    deg_out = np.bincount(src, minlength=N).astype(np.float32)
    deg_in = np.bincount(dst, minlength=N).astype(np.float32)

    # --- dst side: serpentine-deal nodes (by in-degree, descending) into the
    # C*NG (core, group) bins so every bin's total in-degree is nearly equal;
    # this equalizes bucket counts across cores, shrinking the shared
    # max-over-cores gather padding. Empty bin slots are masked out of the
    # BN statistics via a per-slot mask tile. ---
    nbins = C * NG
    order_in = np.argsort(-deg_in, kind="stable")
    i = np.arange(N, dtype=np.int64)
    rnd = i // nbins
    posn = i % nbins
    bin_of = np.where(rnd % 2 == 0, posn, nbins - 1 - posn)
    assert rnd.max() < GRP, "serpentine rounds exceed group rows"
    dst_k = np.empty(N, np.int64)
    dst_g = np.empty(N, np.int64)
    dst_p = np.empty(N, np.int64)
    dst_k[order_in] = bin_of // NG
    dst_g[order_in] = bin_of % NG
    dst_p[order_in] = rnd
    # slot -> node map per core (slot = g*GRP + p), -1 for empty slots
    node_of_slot = np.full((C, NG * GRP), -1, np.int64)
    node_of_slot[dst_k, dst_g * GRP + dst_p] = np.arange(N)

    # --- src side: keep only nodes with outgoing edges; assign them to
    # banks with a greedy balance of the (core, bank, group) bucket counts
    # (shrinks the shared max-over-cores padding), mini-batched for speed ---
    active = np.flatnonzero(deg_out > 0)
    n_active = active.shape[0]
    NBANKS = max(1, _ceil_div(n_active, cfg["BANKCAP"]))
    owner = dst_k[dst]
    grp = dst_g[dst]
    kg = owner * NG + grp  # dst bucket id of each edge (C*NG values)

    src_order = np.argsort(src, kind="stable")
    kg_by_src = kg[src_order]
    csr = np.zeros(N + 1, np.int64)
    csr[1:] = np.cumsum(np.bincount(src, minlength=N))
    # high-degree nodes first
    nodes_by_deg = active[np.argsort(-deg_out[active], kind="stable")]
    cnt = np.zeros((NBANKS, C * NG), np.int32)
    bank_fill = np.zeros(NBANKS, np.int64)
    cap = _ceil_div(n_active, NBANKS)
    # soft per-bucket cap at the next 128 multiple of the mean bucket size:
    # buckets then land at (almost) exactly CAPB, so chunk segments align to
    # 128-edge blocks with no padding
    CAPB = _ceil_div(_ceil_div(E, C * NG * NBANKS), 128) * 128
    srcbank = np.zeros(N, np.int8)
    srcrow = np.zeros(N, np.int32)
    BATCHN = 256
    for i0 in range(0, n_active, BATCHN):
        vs = nodes_by_deg[i0 : i0 + BATCHN]
        kg_cat = np.concatenate([kg_by_src[csr[v] : csr[v + 1]] for v in vs])
        lens = (csr[vs + 1] - csr[vs]).astype(np.int64)
        offs = np.zeros(lens.shape[0], np.int64)
        np.cumsum(lens[:-1], out=offs[1:])
        scores = cnt[:, kg_cat] + (cnt[:, kg_cat] >= CAPB) * 100000
        segsum = np.add.reduceat(scores, offs, axis=1)  # [NBANKS, nv]
        segsum = segsum + np.where(bank_fill >= cap, 1 << 30, 0)[:, None]
        bstar = np.argmin(segsum, axis=0)
        for v, b_, o_, l_ in zip(vs, bstar, offs, lens):
            if bank_fill[b_] >= cap:
                b_ = int(np.argmin(bank_fill))
            srcbank[v] = b_
            srcrow[v] = bank_fill[b_]
            bank_fill[b_] += 1
            np.add.at(cnt[b_], kg_cat[o_ : o_ + l_], 1)
    bank_nodes = []
    for b in range(NBANKS):
        bn = np.flatnonzero((srcbank == b) & (deg_out > 0))
        bank_nodes.append(bn[np.argsort(srcrow[bn], kind="stable")])
    TROWS = _ceil_div(int(bank_fill.max()), 256) * 256
    assert TROWS < 32768

    # sort edges by (owner, bank, group) once; per-core slices via search
    bank = srcbank[src].astype(np.int64)
    key = (owner * NBANKS + bank) * NG + grp
    order = np.argsort(key, kind="stable")
    s_src = src[order]
    s_dst = dst[order]
    s_key = key[order]

    P = (
        cnt.reshape(NBANKS, C, NG).max(axis=1).astype(np.int64)
    )  # [NBANKS, NG] shared bucket sizes
    P = np.maximum(P, GRP)  # every run >= 128 so a block spans <= 2 runs

    # slot layout: chunk-major -> for each chunk of GCHUNK groups, for each
    # bank, the chunk's runs packed back to back; each (chunk, bank) segment
    # padded to x128 (fold into its last run). Group g's four bank-runs
    # accumulate into ONE long-lived PSUM chain (start at bank 0, stop at
    # bank NBANKS-1), so no SBUF agg adds are needed.
    GC = cfg.get("GCHUNK", 8)
    chunks = [list(range(c, min(c + GC, NG))) for c in range(0, NG, GC)]
    run_off = np.zeros((NBANKS, NG), np.int64)
    run_end = np.zeros((NBANKS, NG), np.int64)
    batches = []  # (bank, first_block, n_blocks) per (chunk, bank) segment
    pos = 0
    for ch in chunks:
        for b in range(NBANKS):
            assert pos % 128 == 0
            t0 = pos // 128
            for g in ch:
                run_off[b, g] = pos
                pos += int(P[b, g])
            pad = (-pos) % 128
            run_end[b, ch[0] : ch[-1] + 1] = 0  # filled below
            for g in ch:
                run_end[b, g] = run_off[b, g] + P[b, g]
            run_end[b, ch[-1]] += pad  # fold segment pad into last run
            pos += pad
            batches.append((ch[0], b, t0, pos // 128 - t0))
    nidx_tot = int(pos)
    nb_tot = nidx_tot // 128

    # per-block descriptors: which runs touch block t, with chain start/stop
    blocks = [[] for _ in range(nb_tot)]  # t -> [(g, level, start, stop)]
    for b in range(NBANKS):
        for g in range(NG):
            a = int(run_off[b, g])
            z = int(run_end[b, g])
            t_first = a // 128
            t_last = (z - 1) // 128
            for t in range(t_first, t_last + 1):
                level = 1 if (t == t_first and a % 128 != 0) else 0
                blocks[t].append(
                    (
                        g,
                        level,
                        b == 0 and t == t_first,
                        b == NBANKS - 1 and t == t_last,
                    )
                )
    for t, descs in enumerate(blocks):
        assert 1 <= len(descs) <= 2, (t, descs)
        if len(descs) == 2:
            assert descs[0][1] == 0 and descs[1][1] == 1

    # compact index for blocks that carry a level-1 (B) run: the B one-hot
    # matrix is only built for those columns
    bcol = np.full(nb_tot, -1, np.int64)
    nbB = 0
    for t, descs in enumerate(blocks):
        if len(descs) == 2:
            bcol[t] = nbB
            nbB += 1
    nbB = max(nbB, 1)

    # per-core slot data: gather idx + per-slot one-hot offsets (A/B levels)
    bkeys = (
        np.arange(C)[:, None, None] * NBANKS + np.arange(NBANKS)[None, :, None]
    ) * NG + np.arange(NG)[None, None, :]
    starts = np.searchsorted(s_key, bkeys.ravel()).reshape(C, NBANKS, NG)
    ends = np.searchsorted(s_key, bkeys.ravel(), side="right").reshape(C, NBANKS, NG)

    gidx_cores = []
    doffA_cores = []
    doffB_cores = []
    degw_cores = []
    # level of each slot: is the slot's run the A or B entry of its block?
    slot_level = np.zeros(nidx_tot, np.int8)
    for b in range(NBANKS):
        for g in range(NG):
            a = int(run_off[b, g])
            z = int(run_end[b, g])
            t_first = a // 128
            if a % 128 != 0:
                # slots of this run inside its first block are level B
                first_end = min(z, (t_first + 1) * 128)
                slot_level[a:first_end] = 1
    for k in range(C):
        gidx = np.zeros(nidx_tot, np.int16)
        doff = np.full(nidx_tot, 255.0, np.float32)
        degw = np.ones(nidx_tot, np.float32)
        for b in range(NBANKS):
            for g in range(NG):
                s, e = starts[k, b, g], ends[k, b, g]
                ne = e - s
                if ne == 0:
                    continue
                p0 = int(run_off[b, g])
                gidx[p0 : p0 + ne] = srcrow[s_src[s:e]].astype(np.int16)
                doff[p0 : p0 + ne] = dst_p[s_dst[s:e]].astype(np.float32)
                degw[p0 : p0 + ne] = deg_out[s_src[s:e]]
        dA = np.where(slot_level == 0, doff, 255.0)
        dB = np.where(slot_level == 1, doff, 255.0)
        gidx_cores.append(_wrap16(gidx, nidx_tot // 16))
        import ml_dtypes

        doffA_cores.append(
            np.ascontiguousarray(dA.reshape(nb_tot, 128).T)
        )
        dBm = dB.reshape(nb_tot, 128)[bcol >= 0]
        if dBm.shape[0] == 0:
            dBm = np.full((1, 128), 255.0, np.float32)
        doffB_cores.append(np.ascontiguousarray(dBm.T))
        degw_cores.append(
            np.ascontiguousarray(degw.reshape(nb_tot, 128).T).astype(
                ml_dtypes.bfloat16
            )
        )

    CBLK = max(nb for _, _, _, nb in batches)
    MBMAX = max(
        (
            sum(1 for j in range(nb) if len(blocks[t0 + j]) == 2)
            for _, _, t0, nb in batches
        ),
        default=1,
    )
    meta = dict(
        NPC=NPC,
        NG=NG,
        NBANKS=NBANKS,
        bank_nodes=bank_nodes,
        node_of_slot=node_of_slot,
        TROWS=TROWS,
        nidx_tot=nidx_tot,
        nb_tot=nb_tot,
        nbB=nbB,
        bcol=bcol,
        CBLK=CBLK,
        MBMAX=max(MBMAX, 1),
        blocks=blocks,
        batches=batches,
        deg_out=deg_out,
        deg_in=deg_in,
    )
    return meta, gidx_cores, doffA_cores, doffB_cores, degw_cores


def _row_of_slot(NG, GRP):
    """ypad DRAM row of slot g*GRP+p: paired groups write (g//2)*2*GRP + p*2
    + (g%2); an odd trailing group keeps plain (g p) order."""
    NGp = NG - (NG % 2)
    g = np.arange(NG * GRP) // GRP
    p = np.arange(NG * GRP) % GRP
    rows = np.where(
        g < NGp, (g // 2) * 2 * GRP + p * 2 + (g % 2), g * GRP + p
    )
    return rows


def _build_nc(cfg, meta):
    N, IN, OUT, C = cfg["N"], cfg["IN"], cfg["OUT"], cfg["NCORES"]
    GRP, CH = cfg["GRP"], cfg["CH"]
    NPC, NG = meta["NPC"], meta["NG"]
    NBANKS, TROWS = meta["NBANKS"], meta["TROWS"]
    nidx_tot, nb_tot = meta["nidx_tot"], meta["nb_tot"]
    nbB, bcol = meta["nbB"], meta["bcol"]
    blocks, batches = meta["blocks"], meta["batches"]
    XK = _ceil_div(IN, 128)
    CBLK, MBMAX = meta["CBLK"], meta["MBMAX"]
    assert OUT == 128 and GRP == 128
    NPG = TROWS // 256             # pair-groups per bank
    XCOLS = NBANKS * TROWS         # columns of the interleaved x^T inputs

    nc = bacc.Bacc(
        "TRN2", target_bir_lowering=False, debug=False, num_devices=C
    )

    # ---- external inputs ----
    xt = [
        nc.dram_tensor(f"xt{j}", [128, XCOLS], BF16, kind="ExternalInput")
        for j in range(XK)
    ]
    wt = [
        nc.dram_tensor(f"wt{j}", [128, OUT], BF16, kind="ExternalInput")
        for j in range(XK)
    ]
    gidx_d = nc.dram_tensor("gidx", [128, nidx_tot // 16], I16, kind="ExternalInput")
    dofA_d = nc.dram_tensor("dofA", [128, nb_tot], F32, kind="ExternalInput")
    dofB_d = nc.dram_tensor("dofB", [128, nbB], F32, kind="ExternalInput")
    degw_d = nc.dram_tensor("degw", [128, nb_tot], BF16, kind="ExternalInput")
    degi_d = nc.dram_tensor("degi", [128, NG], F32, kind="ExternalInput")
    bt_d = nc.dram_tensor("bt", [128, OUT], F32, kind="ExternalInput")
    iota_d = nc.dram_tensor("iota", [128, GRP], BF16, kind="ExternalInput")
    gm_d = nc.dram_tensor("gm", [1, OUT], F32, kind="ExternalInput")
    bb_d = nc.dram_tensor("bb", [1, OUT], F32, kind="ExternalInput")
    maskg_d = nc.dram_tensor("maskg", [128, NG], BF16, kind="ExternalInput")
    onesr_d = nc.dram_tensor("onesr", [1, 128], F32, kind="ExternalInput")

    ypad_d = nc.dram_tensor("ypad", [NG * GRP, OUT], BF16, kind="ExternalOutput")

    with tile.TileContext(nc) as tc:
        with (
            tc.tile_pool(name="const", bufs=1) as cpool,
            tc.tile_pool(name="dram", bufs=1, space="DRAM") as dpool,
            tc.tile_pool(name="xw", bufs=6) as xwp,
            tc.tile_pool(name="hch", bufs=4) as hcp,
            tc.tile_pool(name="gath", bufs=6) as gpool,
            tc.tile_pool(name="monehot", bufs=12) as mpool,
            tc.tile_pool(name="etmp", bufs=4) as epool,
            tc.tile_pool(name="bps", bufs=2, space="PSUM") as bpp,
            tc.tile_pool(name="psum", bufs=4, space="PSUM") as ppool,
            tc.tile_pool(name="pstat", bufs=1, space="PSUM") as pspool,
        ):
            # ---- constants / small tiles ----
            wts = [cpool.tile([128, OUT], BF16, name=f"wt_s{j}") for j in range(XK)]
            degw_t = cpool.tile([128, nb_tot], BF16)
            degi_t = cpool.tile([128, NG], F32)
            nw_t = cpool.tile([128, nb_tot], F32)
            ndst_t = cpool.tile([128, NG], F32)
            bt_t = cpool.tile([128, OUT], F32)
            iota_t = cpool.tile([128, GRP], BF16)
            gm_t = cpool.tile([1, OUT], F32)
            bb_t = cpool.tile([1, OUT], F32)
            maskg_t = cpool.tile([128, NG], BF16)
            onesr_t = cpool.tile([1, 128], F32)
            gidx_t = cpool.tile([128, nidx_tot // 16], I16)
            dofA_t = cpool.tile([128, nb_tot], F32)
            dofB_t = cpool.tile([128, nbB], F32)
            agg_t = cpool.tile([128, NG, OUT], BF16, name="agg")

            for j in range(XK):
                nc.sync.dma_start(wts[j][:], wt[j][:])
            nc.sync.dma_start(degw_t[:], degw_d[:])
            nc.sync.dma_start(degi_t[:], degi_d[:])
            nc.sync.dma_start(bt_t[:], bt_d[:])
            nc.sync.dma_start(iota_t[:], iota_d[:])
            nc.sync.dma_start(gm_t[:], gm_d[:])
            nc.sync.dma_start(bb_t[:], bb_d[:])
            nc.sync.dma_start(maskg_t[:], maskg_d[:])
            nc.sync.dma_start(onesr_t[:], onesr_d[:])

            # norms: rsqrt(max(deg, 1))
            for deg_t, norm_t in (
                (degw_t, nw_t),
                (degi_t, ndst_t),
            ):
                nc.vector.tensor_scalar(
                    norm_t[:], deg_t[:], 1.0, None, op0=mybir.AluOpType.max
                )
                nc.vector.reciprocal(norm_t[:], norm_t[:])
                nc.scalar.activation(
                    norm_t[:], norm_t[:], mybir.ActivationFunctionType.Sqrt
                )

            # internal DRAM: per-bank h tables + BN stats exchange buffers
            h_banks = [
                dpool.tile([TROWS, OUT], BF16, name=f"h_bank{b}")
                for b in range(NBANKS)
            ]
            stats_in = dpool.tile([1, 2 * OUT], F32)
            stats_out = dpool.tile([C, 2 * OUT], F32, addr_space="Shared")

            # ---- stage B: h = (x @ W) * norm_src for ALL nodes, by bank ----
            for b in range(NBANKS):
                hview = h_banks[b][:].rearrange(
                    "(c p k) f -> p c k f", p=128, k=2
                )
                for c0 in range(0, NPG, CH):
                    cw = min(CH, NPG - c0)
                    xts = []
                    for j in range(XK):
                        xtile = xwp.tile([128, CH * 256], BF16, tag=f"xt{j}")
                        nc.sync.dma_start(
                            xtile[:, : cw * 256],
                            xt[j][
                                :,
                                b * TROWS
                                + c0 * 256 : b * TROWS
                                + (c0 + cw) * 256,
                            ],
                        )
                        xts.append(xtile)
                    hchunk = hcp.tile([128, CH, 2, OUT], BF16, tag="hch")
                    for i in range(0, cw, 2):
                        iw = min(2, cw - i)
                        ps = bpp.tile([128, 2, 2, OUT], F32, tag="bps")
                        for i2 in range(iw):
                            for par in range(2):
                                for j in range(XK):
                                    nc.tensor.matmul(
                                        ps[:, i2, par, :],
                                        xts[j][
                                            :,
                                            (i + i2) * 256
                                            + par * 128 : (i + i2) * 256
                                            + par * 128
                                            + 128,
                                        ],
                                        wts[j][:],
                                        start=(j == 0),
                                        stop=(j == XK - 1),
                                    )
                        nc.scalar.activation(
                            hchunk[:, i : i + iw, :, :],
                            ps[:, :iw, :, :],
                            mybir.ActivationFunctionType.Copy,
                        )
                    # issued from the ACT sequencer: keeps the SP queue free
                    # for x loads (no head-of-line blocking behind scales)
                    nc.scalar.dma_start(
                        hview[:, c0 : c0 + cw, :, :], hchunk[:, :cw, :, :]
                    )
                if b == 0:
                    # the big gather-index tables are first needed by stage D
                    # (after bank 0 of B): loading them here, behind bank 0's
                    # ACT-queue writes, keeps the startup x-load path clear
                    # and fills later B-phase DMA stalls instead
                    nc.scalar.dma_start(gidx_t[:], gidx_d[:])
                    nc.scalar.dma_start(dofA_t[:], dofA_d[:])
                    nc.scalar.dma_start(dofB_t[:], dofB_d[:])

            # ---- stage D: gather + one-hot matmul segmented sum; group
            # chains accumulate across all 4 banks in PSUM, then stage E
            # (relu(psum*norm_dst + b), BN partials) fires at chain stop ----
            ps_sum_t = pspool.tile([1, OUT], F32, name="ps_sum_t")
            ps_sq_t = pspool.tile([1, OUT], F32, name="ps_sq_t")
            ps_sum = ps_sum_t[:]
            ps_sq = ps_sq_t[:]
            open_ps = {}       # g -> psum accumulation tile of its chain

            def estage(g, ps_run):
                tmp = epool.tile([128, OUT], F32, tag="etmp")
                nc.vector.scalar_tensor_tensor(
                    tmp[:],
                    ps_run[:],
                    ndst_t[:, g : g + 1],
                    bt_t[:],
                    op0=mybir.AluOpType.mult,
                    op1=mybir.AluOpType.add,
                )
                nc.scalar.activation(
                    agg_t[:, g, :], tmp[:], mybir.ActivationFunctionType.Relu
                )
                ones = maskg_t[:, g : g + 1]
                nc.tensor.matmul(
                    ps_sum,
                    ones,
                    agg_t[:, g, :],
                    start=(g == 0),
                    stop=(g == NG - 1),
                )
                sq = epool.tile([128, OUT], BF16, tag="esq")
                nc.scalar.activation(
                    sq[:],
                    agg_t[:, g, :],
                    mybir.ActivationFunctionType.Square,
                )
                nc.tensor.matmul(
                    ps_sq,
                    ones,
                    sq[:],
                    start=(g == 0),
                    stop=(g == NG - 1),
                )

            # gather emission order: hoist a few early bank-0 gathers (their
            # h table is ready first) so the DMA engines have gather work
            # while stage B still computes the later banks; hoisted tiles use
            # a dedicated pool so main-pool buffer rotation stays in
            # consumption order (no WAR deadlock)
            # wavefront hoist: bank-b gathers become runnable once stage B
            # finishes bank b, so pre-emit a few from each earlier bank to
            # cover the DMA stalls at later banks' B boundaries
            nchunks = len(batches) // NBANKS
            hoist_ids = [
                ci_pos * NBANKS + b_
                for ci_pos, b_ in
                [(1, 0), (2, 0), (3, 0), (4, 0)]
                if ci_pos < nchunks and b_ < NBANKS
            ]
            gt_of = {}

            def emit_gather(i):
                ci, bnk, t0, nblk = batches[i]
                pool_tag = ("Gh" if i in hoist_ids else "G")
                pool_bufs = gpool
                Gt = pool_bufs.tile(
                    [128, CBLK, OUT], BF16, tag=pool_tag, name="Gt"
                )
                nc.gpsimd.dma_gather(
                    Gt[:, :nblk, :],
                    h_banks[bnk][:],
                    gidx_t[:, t0 * 8 : (t0 + nblk) * 8],
                    nblk * 128,
                    nblk * 128,
                    OUT,
                    single_packet=False,
                )
                gt_of[i] = Gt

            main_order = [
                i for i in range(len(batches)) if i not in hoist_ids
            ]
            LOOKAHEAD = 3  # main-pool gathers in flight beyond the current
            mptr = 0

            def pump(n):
                nonlocal mptr
                for _ in range(n):
                    if mptr < len(main_order):
                        emit_gather(main_order[mptr])
                        mptr += 1

            pump(1)
            for i in hoist_ids:
                emit_gather(i)
            pump(LOOKAHEAD)

            for bi, (ci, bnk, t0, nblk) in enumerate(batches):
                Gt = gt_of[bi]
                if bi not in hoist_ids:
                    pump(1)
                for j in range(nblk):
                    t = t0 + j
                    for g, level, is_start, is_stop in blocks[t]:
                        if is_start:
                            open_ps[g] = ppool.tile(
                                [128, OUT], F32, tag="aggps", name="ps_run"
                            )
                        # one-hot column with norm_src fused in: the
                        # two-scalar tensor_scalar hits the DVE 4x mode
                        dof_col = (
                            dofB_t[:, bcol[t] : bcol[t] + 1]
                            if level == 1
                            else dofA_t[:, t : t + 1]
                        )
                        Mt = mpool.tile([128, GRP], BF16, tag="M", name="Mt")
                        nc.vector.tensor_scalar(
                            Mt[:],
                            iota_t[:],
                            dof_col,
                            nw_t[:, t : t + 1],
                            op0=mybir.AluOpType.is_equal,
                            op1=mybir.AluOpType.mult,
                        )
                        nc.tensor.matmul(
                            open_ps[g][:],
                            Mt[:],
                            Gt[:, j, :],
                            start=is_start,
                            stop=is_stop,
                        )
                        if is_stop:
                            estage(g, open_ps.pop(g))
            assert not open_ps

            # ---- stage F: AllReduce BN stats; build affine S/T tiles ----
            st_sb = cpool.tile([1, 2 * OUT], F32)
            nc.vector.tensor_copy(st_sb[:, 0:OUT], ps_sum)
            nc.vector.tensor_copy(st_sb[:, OUT : 2 * OUT], ps_sq)
            nc.sync.dma_start(stats_in[:], st_sb[:])
            # AllGather + local reduce: ~half the cost of an AllReduce for
            # this tiny (1 KB) payload
            nc.gpsimd.collective_compute(
                "AllGather",
                mybir.AluOpType.bypass,
                replica_groups=[list(range(C))],
                ins=[stats_in[:]],
                outs=[stats_out[:]],
            )
            st_all = cpool.tile([C, 2 * OUT], F32)
            nc.sync.dma_start(st_all[:], stats_out[:])
            st_red = cpool.tile([C, 2 * OUT], F32)
            nc.gpsimd.partition_all_reduce(
                st_red[:], st_all[:], C, bass_isa.ReduceOp.add
            )
            muex = cpool.tile([1, 2 * OUT], F32)
            var = cpool.tile([1, OUT], F32)
            srow = cpool.tile([1, OUT], F32)
            trow = cpool.tile([1, OUT], F32)
            inv_n = 1.0 / float(N)
            nc.scalar.activation(
                muex[:],
                st_red[0:1, :],
                mybir.ActivationFunctionType.Copy,
                scale=inv_n,
            )
            mu = muex[0:1, 0:OUT]
            ex2 = muex[0:1, OUT : 2 * OUT]
            nc.scalar.activation(var[:], mu, mybir.ActivationFunctionType.Square)
            nc.vector.tensor_sub(var[:], ex2, var[:])
            # var <- rsqrt(var + eps) (ACT Rsqrt is banned for accuracy)
            nc.scalar.activation(
                var[:],
                var[:],
                mybir.ActivationFunctionType.Copy,
                bias=float(cfg["EPS"]),
            )
            nc.vector.reciprocal(var[:], var[:])
            nc.scalar.activation(var[:], var[:], mybir.ActivationFunctionType.Sqrt)
            nc.vector.tensor_mul(srow[:], gm_t[:], var[:])
            nc.vector.tensor_mul(trow[:], mu, srow[:])
            nc.vector.tensor_sub(trow[:], bb_t[:], trow[:])

            S_t = cpool.tile([128, OUT], BF16)
            T_t = cpool.tile([128, OUT], BF16)
            srow_b = cpool.tile([1, OUT], BF16)
            trow_b = cpool.tile([1, OUT], BF16)
            nc.scalar.activation(
                srow_b[:], srow[:], mybir.ActivationFunctionType.Copy
            )
            nc.scalar.activation(
                trow_b[:], trow[:], mybir.ActivationFunctionType.Copy
            )
            nc.gpsimd.partition_broadcast(S_t[:], srow_b[:])
            nc.gpsimd.partition_broadcast(T_t[:], trow_b[:])

            # ---- stage G: y = relu_h * S + T (in place, bf16), chunked
            # writes so the output DMA overlaps the affine math. DRAM row
            # (g//2)*256 + p*2 + (g%2) pairs two adjacent groups of one
            # partition into 512B descriptors (slot->node map unpermutes). ----
            NGp = NG - (NG % 2)  # paired groups; odd tail written separately
            ypad_view = ypad_d[:][0 : NGp * GRP, :].rearrange(
                "(q p k) f -> p q k f", p=128, k=2
            )
            agg_pv = agg_t[:, 0:NGp, :].rearrange("p (q k) f -> p q k f", k=2)
            YC = 8
            ychunks = [(0, min(4, NG))] + [
                (g0, min(YC, NG - g0)) for g0 in range(4, NG, YC)
            ]
            for g0, gw in ychunks:
                S_bc = (
                    S_t[:]
                    .rearrange("p (o f) -> p o f", o=1)
                    .to_broadcast((128, gw, OUT))
                )
                T_bc = (
                    T_t[:]
                    .rearrange("p (o f) -> p o f", o=1)
                    .to_broadcast((128, gw, OUT))
                )
                nc.vector.tensor_tensor(
                    agg_t[:, g0 : g0 + gw, :],
                    agg_t[:, g0 : g0 + gw, :],
                    S_bc,
                    op=mybir.AluOpType.mult,
                )
                nc.vector.tensor_tensor(
                    agg_t[:, g0 : g0 + gw, :],
                    agg_t[:, g0 : g0 + gw, :],
                    T_bc,
                    op=mybir.AluOpType.add,
                )
                gp = min(g0 + gw, NGp)
                if gp > g0:
                    nc.scalar.dma_start(
                        ypad_view[:, g0 // 2 : gp // 2, :, :],
                        agg_pv[:, g0 // 2 : gp // 2, :, :],
                    )
                if g0 + gw > NGp:  # odd tail group, plain (g p) layout
                    nc.scalar.dma_start(
                        ypad_d[:]
                        .rearrange("(g p) f -> p g f", p=128)[
                            :, NGp : NG, :
                        ],
                        agg_t[:, NGp:NG, :],
                    )

    nc.compile()
    return nc


def _host_inputs(
    cfg, meta, x, W, b, gamma, beta, gidx_cores, doffA_cores, doffB_cores, degw_cores
):
    import ml_dtypes

    N, IN, OUT, C = cfg["N"], cfg["IN"], cfg["OUT"], cfg["NCORES"]
    GRP = cfg["GRP"]
    NPC, NG = meta["NPC"], meta["NG"]
    NBANKS, TROWS = meta["NBANKS"], meta["TROWS"]
    bank_nodes = meta["bank_nodes"]
    XK = _ceil_div(IN, 128)
    NPG = TROWS // 256
    XCOLS = NBANKS * TROWS

    # column c of xt{j}: b = c//TROWS, r = c%TROWS; pg = r//256, rem = r%256,
    # parity = rem//128, p = rem%128 -> table row pg*256 + 2p + parity;
    # table row r of bank b holds node bank_nodes[b][r] (compacted/balanced)
    c = np.arange(XCOLS, dtype=np.int64)
    bnk = c // TROWS
    r = c % TROWS
    pg = r // 256
    rem = r % 256
    parity = rem // 128
    p = rem % 128
    localrow = pg * 256 + 2 * p + parity
    node_safe = np.zeros(XCOLS, np.int64)
    valid = np.zeros(XCOLS, np.float32)
    for bb_ in range(NBANKS):
        nrows = bank_nodes[bb_].shape[0]
        m = (bnk == bb_) & (localrow < nrows)
        node_safe[m] = bank_nodes[bb_][localrow[m]]
        valid[m] = 1.0

    xT = np.ascontiguousarray(np.asarray(x, np.float32).T)  # [IN, N]
    Wn = np.asarray(W, np.float32)
    xt_list = []
    for j in range(XK):
        sl = xT[j * 128 : (j + 1) * 128, :]  # [128, N]
        arr = sl[:, node_safe] * valid[None, :]
        xt_list.append(np.ascontiguousarray(arr).astype(ml_dtypes.bfloat16))
    wt_list = [
        np.ascontiguousarray(Wn[j * 128 : (j + 1) * 128, :]).astype(
            ml_dtypes.bfloat16
        )
        for j in range(XK)
    ]

    iota = np.tile(
        np.arange(GRP, dtype=np.float32)[None, :], (128, 1)
    ).astype(ml_dtypes.bfloat16)
    bt = np.tile(np.asarray(b, np.float32)[None, :], (128, 1))
    onesr = np.ones((1, 128), np.float32)
    gm = np.asarray(gamma, np.float32)[None, :]
    bb = np.asarray(beta, np.float32)[None, :]
    node_of_slot = meta["node_of_slot"]
    deg_in = meta["deg_in"]

    in_maps = []
    for k in range(C):
        slots = node_of_slot[k]  # [NG*GRP], -1 = empty
        occ = slots >= 0
        degi_k = np.where(occ, deg_in[np.where(occ, slots, 0)], 1.0).astype(
            np.float32
        )
        # [128, NG] tile-major: entry (p, g) = value at slot g*GRP + p
        degi_t = np.ascontiguousarray(degi_k.reshape(NG, GRP).T)
        mask_t = np.ascontiguousarray(
            occ.astype(np.float32).reshape(NG, GRP).T
        ).astype(ml_dtypes.bfloat16)
        im = {
            "gidx": gidx_cores[k],
            "dofA": doffA_cores[k],
            "dofB": doffB_cores[k],
            "degw": degw_cores[k],
            "degi": degi_t,
            "maskg": mask_t,
            "bt": bt,
            "iota": iota,
            "gm": gm,
            "bb": bb,
            "onesr": onesr,
        }
        for j in range(XK):
            im[f"xt{j}"] = xt_list[j]
            im[f"wt{j}"] = wt_list[j]
        in_maps.append(im)
    return in_maps


def kernel(x, src, dst, W, b, gamma, beta):
    global LAST_RESULTS, LAST_NC, LAST_RUN_S
    cfg = CFG
    N, E, IN, OUT, C = cfg["N"], cfg["E"], cfg["IN"], cfg["OUT"], cfg["NCORES"]
    assert x.shape == (N, IN) and W.shape == (IN, OUT)
    assert src.shape == (E,) and dst.shape == (E,)

    meta, gidx_cores, doffA_cores, doffB_cores, degw_cores = _preprocess(
        cfg, src, dst
    )
    NPC = meta["NPC"]

    nc = _build_nc(cfg, meta)
    in_maps = _host_inputs(
        cfg,
        meta,
        x,
        W,
        b,
        gamma,
        beta,
        gidx_cores,
        doffA_cores,
        doffB_cores,
        degw_cores,
    )

    if cfg.get("SIM"):
        from concourse.bass_interp import MultiCoreSim

        sim = MultiCoreSim(nc, num_cores=C)
        for k, core_sim in sim.cores.items():
            for name, val in in_maps[k].items():
                core_sim.tensor(name)[:] = val
        sim.simulate()
        y = np.empty((N, OUT), np.float32)
        node_of_slot = meta["node_of_slot"]
        rows = _row_of_slot(meta["NG"], cfg["GRP"])
        for k in range(C):
            slots = node_of_slot[k]
            occ = slots >= 0
            yp = np.asarray(sim.cores[k].tensor("ypad"), np.float32)
            y[slots[occ]] = yp[rows[occ]]
        return y

    LAST_NC = nc
    import time as _time

    _t0 = _time.time()
    res = bass_utils.run_bass_kernel_spmd(
        nc,
        in_maps,
        core_ids=list(range(C)),
        trace=cfg.get("TRACE", False),
    )
    LAST_RUN_S = _time.time() - _t0
    LAST_RESULTS = res

    y = np.empty((N, OUT), np.float32)
    node_of_slot = meta["node_of_slot"]
    rows = _row_of_slot(meta["NG"], cfg["GRP"])
    for k in range(C):
        slots = node_of_slot[k]
        occ = slots >= 0
        yp = np.asarray(res.results[k]["ypad"], np.float32)
        y[slots[occ]] = yp[rows[occ]]
    return y



# revision 6
# speedup vs baseline: 1.0444x; 1.0444x over previous
"""GCN block (GraphConv + BatchNorm1d + ReLU) on 8 Trainium2 NeuronCores.

Strategy: every core computes h = x @ W for ALL nodes (replicated matmul) so
no AllGather of node features is needed -- the extra x reads (51 MB vs 6.4 MB
per core) are far cheaper than a 25 MB collective. Nodes are partitioned by
dst across cores for the aggregation; W/b/gamma/beta are replicated and only
the 1 KB BN batch statistics cross cores (AllGather + on-device reduce).

Host-side preprocessing (integer index bookkeeping only):
  * dst side: nodes are serpentine-dealt (by in-degree, descending) into the
    C*NG (core, 128-row group) bins, equalizing per-bin in-degree totals;
    empty bin slots are masked out of the BN stats by a per-slot mask tile,
    and the output rows are unpermuted on the host.
  * src side: nodes with outgoing edges are greedily assigned to int16-sized
    banks balancing every (core, bank, group) bucket count. Together the two
    permutations shrink the shared (SPMD max-over-cores) gather padding to
    ~1% of the edge count.

Device pipeline, per core k:
  B. h = x @ W for all N nodes (bf16) written to per-bank HBM tables; the
     SBUF->HBM write uses a paired-row layout (partition p holds table rows
     2p/2p+1 of each 256-row group) so DMA descriptors are 512 B, not 256 B.
  D. Per (4-group chunk, bank): dma_gather the chunk's edges' h[src] rows
     (bf16, ~30 blocks of 128 edges per call; a few early-bank gathers are
     hoisted ahead in the Pool FIFO to fill stage-B DMA stalls) and
     segment-sum them with one-hot matmuls M^T @ G. Each group's chain
     accumulates across all banks in its own PSUM bank (accumulation groups
     are PSUM-bank-scoped). M columns are built per 128-edge block by one
     DVE tensor_scalar (iota == dst_offset) * rsqrt(deg_out[src]) -- the
     two-scalar form hits the DVE 4x perf mode and folds the source-side
     norm in for free. A block straddling two buckets runs two matmuls
     (offsets relative to each bucket; non-members hold 255 -> zero column).
  E. At chain stop: relu(psum * rsqrt(clip(deg_in,1)) + bias) -> agg (bf16);
     masked BN partial sums accumulate on two PSUM chains.
  F. AllGather of the 1 KB stats + partition_all_reduce; build affine S/T.
  G. y = relu_h * S + T in place (bf16, DVE 2x), chunked DMA out.

All floating-point math runs on device; the host only does integer
bucketing/permutations, degree counting (bincount), and dtype casts.
"""
import sys

sys.path.insert(0, "/opt/trn_rl_repo")

import numpy as np

import concourse.bacc as bacc
import concourse.bass as bass
import concourse.bass_isa as bass_isa
import concourse.mybir as mybir
import concourse.tile as tile
from concourse import bass_utils

F32 = mybir.dt.float32
BF16 = mybir.dt.bfloat16
I16 = mybir.dt.int16

CFG = dict(
    N=100000,
    E=1600000,
    IN=256,
    OUT=128,
    NCORES=8,
    GRP=128,          # dst nodes per segment group (= psum partition dim)
    BANKCAP=32512,    # max rows per src bank (int16 gather-index limit)
    GCHUNK=4,         # dst groups per chunk (concurrent PSUM accum chains;
                      # each chain needs its own PSUM bank)
    CH=4,            # x@W chunk size in 256-node pair-groups
    EPS=1e-5,
    TRACE=False,
)

LAST_RESULTS = None  # set by kernel() for test harness introspection
LAST_NC = None
LAST_RUN_S = None


def _ceil_div(a, b):
    return (a + b - 1) // b


def _wrap16(idx, ncols):
    """int16 idx list -> [128, ncols] tile: idx i at [i%16, i//16], replicated
    8x across the 16-partition groups (one copy per GpSimd Q7 core)."""
    n = idx.shape[0]
    assert n == ncols * 16
    w = np.ascontiguousarray(idx.reshape(ncols, 16).T)
    return np.tile(w, (8, 1))


def _preprocess(cfg, src, dst):
    """Bucket edges by (owner core, src bank, dst group); build per-core
    gather-index / dst-offset arrays and the shared block structure."""
    N, E = cfg["N"], cfg["E"]
    C, GRP = cfg["NCORES"], cfg["GRP"]
    NPC = N // C
    NG = _ceil_div(NPC, GRP)

    src = src.astype(np.int64)
    dst = dst.astype(np.int64)
    deg_out = np.bincount(src, minlength=N).astype(np.float32)
    deg_in = np.bincount(dst, minlength=N).astype(np.float32)

    # --- dst side: serpentine-deal nodes (by in-degree, descending) into the
    # C*NG (core, group) bins so every bin's total in-degree is nearly equal;
    # this equalizes bucket counts across cores, shrinking the shared
    # max-over-cores gather padding. Empty bin slots are masked out of the
    # BN statistics via a per-slot mask tile. ---
    nbins = C * NG
    order_in = np.argsort(-deg_in, kind="stable")
    i = np.arange(N, dtype=np.int64)
    rnd = i // nbins
    posn = i % nbins
    bin_of = np.where(rnd % 2 == 0, posn, nbins - 1 - posn)
    assert rnd.max() < GRP, "serpentine rounds exceed group rows"
    dst_k = np.empty(N, np.int64)
    dst_g = np.empty(N, np.int64)
    dst_p = np.empty(N, np.int64)
    dst_k[order_in] = bin_of // NG
    dst_g[order_in] = bin_of % NG
    dst_p[order_in] = rnd
    # slot -> node map per core (slot = g*GRP + p), -1 for empty slots
    node_of_slot = np.full((C, NG * GRP), -1, np.int64)
    node_of_slot[dst_k, dst_g * GRP + dst_p] = np.arange(N)

    # --- src side, PER CORE: each core's h table only holds the distinct
    # srcs of ITS edges (~86.5k of 100k), cutting the replicated x reads and
    # table writes by ~14%. Each core gets its own bank/row assignment (and
    # so its own xt column permutation); only the run sizes P[b,g] (max over
    # cores) and TROWS are shared compile-time quantities. ---
    owner = dst_k[dst]
    grp = dst_g[dst]

    # edges sorted by (owner, src): per-core CSR over srcs for the greedy
    eorder = np.argsort(owner * N + src, kind="stable")
    o_s = owner[eorder]
    s_s = src[eorder]
    g_s = grp[eorder]
    core_starts = np.searchsorted(o_s, np.arange(C + 1))

    per_core = []
    max_active = 0
    for k in range(C):
        a, b = core_starts[k], core_starts[k + 1]
        degk = np.bincount(s_s[a:b], minlength=N)
        actk = np.flatnonzero(degk)
        per_core.append((s_s[a:b], g_s[a:b], degk, actk))
        max_active = max(max_active, actk.size)
    NBANKS = max(1, _ceil_div(max_active, cfg["BANKCAP"]))
    # soft per-bucket cap at the next 128 multiple of the mean bucket size
    CAPB = _ceil_div(_ceil_div(E, C * NG * NBANKS), 128) * 128

    srcbank = np.zeros((C, N), np.int8)
    srcrow = np.zeros((C, N), np.int32)
    bank_nodes_cores = []
    cnt_max = np.zeros((NBANKS, NG), np.int64)
    fill_max = 0
    BATCHN = 256
    for k in range(C):
        sk, gk, degk, actk = per_core[k]
        csr = np.zeros(N + 1, np.int64)
        csr[1:] = np.cumsum(degk)
        nodes_by_deg = actk[np.argsort(-degk[actk], kind="stable")]
        cnt = np.zeros((NBANKS, NG), np.int32)
        bank_fill = np.zeros(NBANKS, np.int64)
        cap = _ceil_div(actk.size, NBANKS)
        sb = srcbank[k]
        sr = srcrow[k]
        for i0 in range(0, actk.size, BATCHN):
            vs = nodes_by_deg[i0 : i0 + BATCHN]
            kg_cat = np.concatenate([gk[csr[v] : csr[v + 1]] for v in vs])
            lens = (csr[vs + 1] - csr[vs]).astype(np.int64)
            offs = np.zeros(lens.shape[0], np.int64)
            np.cumsum(lens[:-1], out=offs[1:])
            scores = cnt[:, kg_cat] + (cnt[:, kg_cat] >= CAPB) * 100000
            segsum = np.add.reduceat(scores, offs, axis=1)  # [NBANKS, nv]
            segsum = segsum + np.where(bank_fill >= cap, 1 << 30, 0)[:, None]
            bstar = np.argmin(segsum, axis=0)
            for v, b_, o_, l_ in zip(vs, bstar, offs, lens):
                if bank_fill[b_] >= cap:
                    b_ = int(np.argmin(bank_fill))
                sb[v] = b_
                sr[v] = bank_fill[b_]
                bank_fill[b_] += 1
                np.add.at(cnt[b_], kg_cat[o_ : o_ + l_], 1)
        bn_list = []
        for b in range(NBANKS):
            bn = np.flatnonzero((sb == b) & (degk > 0))
            bn_list.append(bn[np.argsort(sr[bn], kind="stable")])
        bank_nodes_cores.append(bn_list)
        cnt_max = np.maximum(cnt_max, cnt)
        fill_max = max(fill_max, int(bank_fill.max()))
    TROWS = _ceil_div(fill_max, 256) * 256
    assert TROWS < 32768

    # sort edges by (owner, bank, group) once; per-core slices via search
    bank = srcbank[owner, src].astype(np.int64)
    key = (owner * NBANKS + bank) * NG + grp
    order = np.argsort(key, kind="stable")
    s_src = src[order]
    s_dst = dst[order]
    s_key = key[order]

    P = np.maximum(cnt_max, GRP)  # [NBANKS, NG] shared bucket sizes;
    # every run >= 128 so a block spans <= 2 runs

    # slot layout: chunk-major -> for each chunk of GCHUNK groups, for each
    # bank, the chunk's runs packed back to back; each (chunk, bank) segment
    # padded to x128 (fold into its last run). Group g's four bank-runs
    # accumulate into ONE long-lived PSUM chain (start at bank 0, stop at
    # bank NBANKS-1), so no SBUF agg adds are needed.
    GC = cfg.get("GCHUNK", 8)
    chunks = [list(range(c, min(c + GC, NG))) for c in range(0, NG, GC)]
    run_off = np.zeros((NBANKS, NG), np.int64)
    run_end = np.zeros((NBANKS, NG), np.int64)
    batches = []  # (bank, first_block, n_blocks) per (chunk, bank) segment
    pos = 0
    for ch in chunks:
        for b in range(NBANKS):
            assert pos % 128 == 0
            t0 = pos // 128
            for g in ch:
                run_off[b, g] = pos
                pos += int(P[b, g])
            pad = (-pos) % 128
            run_end[b, ch[0] : ch[-1] + 1] = 0  # filled below
            for g in ch:
                run_end[b, g] = run_off[b, g] + P[b, g]
            run_end[b, ch[-1]] += pad  # fold segment pad into last run
            pos += pad
            batches.append((ch[0], b, t0, pos // 128 - t0))
    nidx_tot = int(pos)
    nb_tot = nidx_tot // 128

    # per-block descriptors: which runs touch block t, with chain start/stop
    blocks = [[] for _ in range(nb_tot)]  # t -> [(g, level, start, stop)]
    for b in range(NBANKS):
        for g in range(NG):
            a = int(run_off[b, g])
            z = int(run_end[b, g])
            t_first = a // 128
            t_last = (z - 1) // 128
            for t in range(t_first, t_last + 1):
                level = 1 if (t == t_first and a % 128 != 0) else 0
                blocks[t].append(
                    (
                        g,
                        level,
                        b == 0 and t == t_first,
                        b == NBANKS - 1 and t == t_last,
                    )
                )
    for t, descs in enumerate(blocks):
        assert 1 <= len(descs) <= 2, (t, descs)
        if len(descs) == 2:
            assert descs[0][1] == 0 and descs[1][1] == 1

    # compact index for blocks that carry a level-1 (B) run: the B one-hot
    # matrix is only built for those columns
    bcol = np.full(nb_tot, -1, np.int64)
    nbB = 0
    for t, descs in enumerate(blocks):
        if len(descs) == 2:
            bcol[t] = nbB
            nbB += 1
    nbB = max(nbB, 1)

    # per-core slot data: gather idx + per-slot one-hot offsets (A/B levels)
    bkeys = (
        np.arange(C)[:, None, None] * NBANKS + np.arange(NBANKS)[None, :, None]
    ) * NG + np.arange(NG)[None, None, :]
    starts = np.searchsorted(s_key, bkeys.ravel()).reshape(C, NBANKS, NG)
    ends = np.searchsorted(s_key, bkeys.ravel(), side="right").reshape(C, NBANKS, NG)

    gidx_cores = []
    doffA_cores = []
    doffB_cores = []
    degw_cores = []
    # level of each slot: is the slot's run the A or B entry of its block?
    slot_level = np.zeros(nidx_tot, np.int8)
    for b in range(NBANKS):
        for g in range(NG):
            a = int(run_off[b, g])
            z = int(run_end[b, g])
            t_first = a // 128
            if a % 128 != 0:
                # slots of this run inside its first block are level B
                first_end = min(z, (t_first + 1) * 128)
                slot_level[a:first_end] = 1
    for k in range(C):
        gidx = np.zeros(nidx_tot, np.int16)
        doff = np.full(nidx_tot, 255.0, np.float32)
        degw = np.ones(nidx_tot, np.float32)
        for b in range(NBANKS):
            for g in range(NG):
                s, e = starts[k, b, g], ends[k, b, g]
                ne = e - s
                if ne == 0:
                    continue
                p0 = int(run_off[b, g])
                gidx[p0 : p0 + ne] = srcrow[k][s_src[s:e]].astype(np.int16)
                doff[p0 : p0 + ne] = dst_p[s_dst[s:e]].astype(np.float32)
                degw[p0 : p0 + ne] = deg_out[s_src[s:e]]
        dA = np.where(slot_level == 0, doff, 255.0)
        dB = np.where(slot_level == 1, doff, 255.0)
        gidx_cores.append(_wrap16(gidx, nidx_tot // 16))
        import ml_dtypes

        doffA_cores.append(
            np.ascontiguousarray(dA.reshape(nb_tot, 128).T)
        )
        dBm = dB.reshape(nb_tot, 128)[bcol >= 0]
        if dBm.shape[0] == 0:
            dBm = np.full((1, 128), 255.0, np.float32)
        doffB_cores.append(np.ascontiguousarray(dBm.T))
        degw_cores.append(
            np.ascontiguousarray(degw.reshape(nb_tot, 128).T).astype(
                ml_dtypes.bfloat16
            )
        )

    CBLK = max(nb for _, _, _, nb in batches)
    MBMAX = max(
        (
            sum(1 for j in range(nb) if len(blocks[t0 + j]) == 2)
            for _, _, t0, nb in batches
        ),
        default=1,
    )
    meta = dict(
        NPC=NPC,
        NG=NG,
        NBANKS=NBANKS,
        bank_nodes_cores=bank_nodes_cores,
        node_of_slot=node_of_slot,
        TROWS=TROWS,
        nidx_tot=nidx_tot,
        nb_tot=nb_tot,
        nbB=nbB,
        bcol=bcol,
        CBLK=CBLK,
        MBMAX=max(MBMAX, 1),
        blocks=blocks,
        batches=batches,
        deg_out=deg_out,
        deg_in=deg_in,
    )
    return meta, gidx_cores, doffA_cores, doffB_cores, degw_cores


def _row_of_slot(NG, GRP):
    """ypad DRAM row of slot g*GRP+p: paired groups write (g//2)*2*GRP + p*2
    + (g%2); an odd trailing group keeps plain (g p) order."""
    NGp = NG - (NG % 2)
    g = np.arange(NG * GRP) // GRP
    p = np.arange(NG * GRP) % GRP
    rows = np.where(
        g < NGp, (g // 2) * 2 * GRP + p * 2 + (g % 2), g * GRP + p
    )
    return rows


def _build_nc(cfg, meta):
    N, IN, OUT, C = cfg["N"], cfg["IN"], cfg["OUT"], cfg["NCORES"]
    GRP, CH = cfg["GRP"], cfg["CH"]
    NPC, NG = meta["NPC"], meta["NG"]
    NBANKS, TROWS = meta["NBANKS"], meta["TROWS"]
    nidx_tot, nb_tot = meta["nidx_tot"], meta["nb_tot"]
    nbB, bcol = meta["nbB"], meta["bcol"]
    blocks, batches = meta["blocks"], meta["batches"]
    XK = _ceil_div(IN, 128)
    CBLK, MBMAX = meta["CBLK"], meta["MBMAX"]
    assert OUT == 128 and GRP == 128
    NPG = TROWS // 256             # pair-groups per bank
    XCOLS = NBANKS * TROWS         # columns of the interleaved x^T inputs

    nc = bacc.Bacc(
        "TRN2", target_bir_lowering=False, debug=False, num_devices=C
    )

    # ---- external inputs ----
    xt = [
        nc.dram_tensor(f"xt{j}", [128, XCOLS], BF16, kind="ExternalInput")
        for j in range(XK)
    ]
    wt = [
        nc.dram_tensor(f"wt{j}", [128, OUT], BF16, kind="ExternalInput")
        for j in range(XK)
    ]
    gidx_d = nc.dram_tensor("gidx", [128, nidx_tot // 16], I16, kind="ExternalInput")
    dofA_d = nc.dram_tensor("dofA", [128, nb_tot], F32, kind="ExternalInput")
    dofB_d = nc.dram_tensor("dofB", [128, nbB], F32, kind="ExternalInput")
    degw_d = nc.dram_tensor("degw", [128, nb_tot], BF16, kind="ExternalInput")
    degi_d = nc.dram_tensor("degi", [128, NG], F32, kind="ExternalInput")
    bt_d = nc.dram_tensor("bt", [128, OUT], F32, kind="ExternalInput")
    iota_d = nc.dram_tensor("iota", [128, GRP], BF16, kind="ExternalInput")
    gm_d = nc.dram_tensor("gm", [1, OUT], F32, kind="ExternalInput")
    bb_d = nc.dram_tensor("bb", [1, OUT], F32, kind="ExternalInput")
    maskg_d = nc.dram_tensor("maskg", [128, NG], BF16, kind="ExternalInput")
    onesr_d = nc.dram_tensor("onesr", [1, 128], F32, kind="ExternalInput")

    ypad_d = nc.dram_tensor("ypad", [NG * GRP, OUT], BF16, kind="ExternalOutput")

    with tile.TileContext(nc) as tc:
        with (
            tc.tile_pool(name="const", bufs=1) as cpool,
            tc.tile_pool(name="dram", bufs=1, space="DRAM") as dpool,
            tc.tile_pool(name="xw", bufs=6) as xwp,
            tc.tile_pool(name="hch", bufs=4) as hcp,
            tc.tile_pool(name="gath", bufs=6) as gpool,
            tc.tile_pool(name="monehot", bufs=12) as mpool,
            tc.tile_pool(name="etmp", bufs=4) as epool,
            tc.tile_pool(name="bps", bufs=2, space="PSUM") as bpp,
            tc.tile_pool(name="psum", bufs=4, space="PSUM") as ppool,
            tc.tile_pool(name="pstat", bufs=1, space="PSUM") as pspool,
        ):
            # ---- constants / small tiles ----
            wts = [cpool.tile([128, OUT], BF16, name=f"wt_s{j}") for j in range(XK)]
            degw_t = cpool.tile([128, nb_tot], BF16)
            degi_t = cpool.tile([128, NG], F32)
            nw_t = cpool.tile([128, nb_tot], F32)
            ndst_t = cpool.tile([128, NG], F32)
            bt_t = cpool.tile([128, OUT], F32)
            iota_t = cpool.tile([128, GRP], BF16)
            gm_t = cpool.tile([1, OUT], F32)
            bb_t = cpool.tile([1, OUT], F32)
            maskg_t = cpool.tile([128, NG], BF16)
            onesr_t = cpool.tile([1, 128], F32)
            gidx_t = cpool.tile([128, nidx_tot // 16], I16)
            dofA_t = cpool.tile([128, nb_tot], F32)
            dofB_t = cpool.tile([128, nbB], F32)
            agg_t = cpool.tile([128, NG, OUT], BF16, name="agg")

            for j in range(XK):
                nc.sync.dma_start(wts[j][:], wt[j][:])
            nc.sync.dma_start(degw_t[:], degw_d[:])
            nc.sync.dma_start(degi_t[:], degi_d[:])
            nc.sync.dma_start(bt_t[:], bt_d[:])
            nc.sync.dma_start(iota_t[:], iota_d[:])
            nc.sync.dma_start(gm_t[:], gm_d[:])
            nc.sync.dma_start(bb_t[:], bb_d[:])
            nc.sync.dma_start(maskg_t[:], maskg_d[:])
            nc.sync.dma_start(onesr_t[:], onesr_d[:])

            # norms: rsqrt(max(deg, 1))
            for deg_t, norm_t in (
                (degw_t, nw_t),
                (degi_t, ndst_t),
            ):
                nc.vector.tensor_scalar(
                    norm_t[:], deg_t[:], 1.0, None, op0=mybir.AluOpType.max
                )
                nc.vector.reciprocal(norm_t[:], norm_t[:])
                nc.scalar.activation(
                    norm_t[:], norm_t[:], mybir.ActivationFunctionType.Sqrt
                )

            # internal DRAM: per-bank h tables + BN stats exchange buffers
            h_banks = [
                dpool.tile([TROWS, OUT], BF16, name=f"h_bank{b}")
                for b in range(NBANKS)
            ]
            stats_in = dpool.tile([1, 2 * OUT], F32)
            stats_out = dpool.tile([C, 2 * OUT], F32, addr_space="Shared")

            # ---- stage B: h = (x @ W) * norm_src for ALL nodes, by bank ----
            for b in range(NBANKS):
                hview = h_banks[b][:].rearrange(
                    "(c p k) f -> p c k f", p=128, k=2
                )
                for c0 in range(0, NPG, CH):
                    cw = min(CH, NPG - c0)
                    xts = []
                    for j in range(XK):
                        xtile = xwp.tile([128, CH * 256], BF16, tag=f"xt{j}")
                        nc.sync.dma_start(
                            xtile[:, : cw * 256],
                            xt[j][
                                :,
                                b * TROWS
                                + c0 * 256 : b * TROWS
                                + (c0 + cw) * 256,
                            ],
                        )
                        xts.append(xtile)
                    hchunk = hcp.tile([128, CH, 2, OUT], BF16, tag="hch")
                    for i in range(0, cw, 2):
                        iw = min(2, cw - i)
                        ps = bpp.tile([128, 2, 2, OUT], F32, tag="bps")
                        for i2 in range(iw):
                            for par in range(2):
                                for j in range(XK):
                                    nc.tensor.matmul(
                                        ps[:, i2, par, :],
                                        xts[j][
                                            :,
                                            (i + i2) * 256
                                            + par * 128 : (i + i2) * 256
                                            + par * 128
                                            + 128,
                                        ],
                                        wts[j][:],
                                        start=(j == 0),
                                        stop=(j == XK - 1),
                                    )
                        nc.scalar.activation(
                            hchunk[:, i : i + iw, :, :],
                            ps[:, :iw, :, :],
                            mybir.ActivationFunctionType.Copy,
                        )
                    # issued from the ACT sequencer: keeps the SP queue free
                    # for x loads (no head-of-line blocking behind scales)
                    nc.scalar.dma_start(
                        hview[:, c0 : c0 + cw, :, :], hchunk[:, :cw, :, :]
                    )
                if b == 0:
                    # the big gather-index tables are first needed by stage D
                    # (after bank 0 of B): loading them here, behind bank 0's
                    # ACT-queue writes, keeps the startup x-load path clear
                    # and fills later B-phase DMA stalls instead
                    nc.scalar.dma_start(gidx_t[:], gidx_d[:])
                    nc.scalar.dma_start(dofA_t[:], dofA_d[:])
                    nc.scalar.dma_start(dofB_t[:], dofB_d[:])

            # ---- stage D: gather + one-hot matmul segmented sum; group
            # chains accumulate across all 4 banks in PSUM, then stage E
            # (relu(psum*norm_dst + b), BN partials) fires at chain stop ----
            ps_sum_t = pspool.tile([1, OUT], F32, name="ps_sum_t")
            ps_sq_t = pspool.tile([1, OUT], F32, name="ps_sq_t")
            ps_sum = ps_sum_t[:]
            ps_sq = ps_sq_t[:]
            open_ps = {}       # g -> psum accumulation tile of its chain

            def estage(g, ps_run):
                tmp = epool.tile([128, OUT], F32, tag="etmp")
                nc.vector.scalar_tensor_tensor(
                    tmp[:],
                    ps_run[:],
                    ndst_t[:, g : g + 1],
                    bt_t[:],
                    op0=mybir.AluOpType.mult,
                    op1=mybir.AluOpType.add,
                )
                nc.scalar.activation(
                    agg_t[:, g, :], tmp[:], mybir.ActivationFunctionType.Relu
                )
                ones = maskg_t[:, g : g + 1]
                nc.tensor.matmul(
                    ps_sum,
                    ones,
                    agg_t[:, g, :],
                    start=(g == 0),
                    stop=(g == NG - 1),
                )
                sq = epool.tile([128, OUT], BF16, tag="esq")
                nc.scalar.activation(
                    sq[:],
                    agg_t[:, g, :],
                    mybir.ActivationFunctionType.Square,
                )
                nc.tensor.matmul(
                    ps_sq,
                    ones,
                    sq[:],
                    start=(g == 0),
                    stop=(g == NG - 1),
                )

            # gather emission order: hoist a few early bank-0 gathers (their
            # h table is ready first) so the DMA engines have gather work
            # while stage B still computes the later banks; hoisted tiles use
            # a dedicated pool so main-pool buffer rotation stays in
            # consumption order (no WAR deadlock)
            # wavefront hoist: bank-b gathers become runnable once stage B
            # finishes bank b, so pre-emit a few from each earlier bank to
            # cover the DMA stalls at later banks' B boundaries
            nchunks = len(batches) // NBANKS
            hoist_ids = [
                ci_pos * NBANKS + b_
                for ci_pos, b_ in
                [(1, 0), (2, 0), (3, 0), (4, 0)]
                if ci_pos < nchunks and b_ < NBANKS
            ]
            gt_of = {}

            def emit_gather(i):
                ci, bnk, t0, nblk = batches[i]
                pool_tag = ("Gh" if i in hoist_ids else "G")
                pool_bufs = gpool
                Gt = pool_bufs.tile(
                    [128, CBLK, OUT], BF16, tag=pool_tag, name="Gt"
                )
                nc.gpsimd.dma_gather(
                    Gt[:, :nblk, :],
                    h_banks[bnk][:],
                    gidx_t[:, t0 * 8 : (t0 + nblk) * 8],
                    nblk * 128,
                    nblk * 128,
                    OUT,
                    single_packet=False,
                )
                gt_of[i] = Gt

            main_order = [
                i for i in range(len(batches)) if i not in hoist_ids
            ]
            LOOKAHEAD = 3  # main-pool gathers in flight beyond the current
            mptr = 0

            def pump(n):
                nonlocal mptr
                for _ in range(n):
                    if mptr < len(main_order):
                        emit_gather(main_order[mptr])
                        mptr += 1

            pump(1)
            for i in hoist_ids:
                emit_gather(i)
            pump(LOOKAHEAD)

            for bi, (ci, bnk, t0, nblk) in enumerate(batches):
                Gt = gt_of[bi]
                if bi not in hoist_ids:
                    pump(1)
                for j in range(nblk):
                    t = t0 + j
                    for g, level, is_start, is_stop in blocks[t]:
                        if is_start:
                            open_ps[g] = ppool.tile(
                                [128, OUT], F32, tag="aggps", name="ps_run"
                            )
                        # one-hot column with norm_src fused in: the
                        # two-scalar tensor_scalar hits the DVE 4x mode
                        dof_col = (
                            dofB_t[:, bcol[t] : bcol[t] + 1]
                            if level == 1
                            else dofA_t[:, t : t + 1]
                        )
                        Mt = mpool.tile([128, GRP], BF16, tag="M", name="Mt")
                        nc.vector.tensor_scalar(
                            Mt[:],
                            iota_t[:],
                            dof_col,
                            nw_t[:, t : t + 1],
                            op0=mybir.AluOpType.is_equal,
                            op1=mybir.AluOpType.mult,
                        )
                        nc.tensor.matmul(
                            open_ps[g][:],
                            Mt[:],
                            Gt[:, j, :],
                            start=is_start,
                            stop=is_stop,
                        )
                        if is_stop:
                            estage(g, open_ps.pop(g))
            assert not open_ps

            # ---- stage F: AllReduce BN stats; build affine S/T tiles ----
            st_sb = cpool.tile([1, 2 * OUT], F32)
            nc.vector.tensor_copy(st_sb[:, 0:OUT], ps_sum)
            nc.vector.tensor_copy(st_sb[:, OUT : 2 * OUT], ps_sq)
            nc.sync.dma_start(stats_in[:], st_sb[:])
            # AllGather + local reduce: ~half the cost of an AllReduce for
            # this tiny (1 KB) payload
            nc.gpsimd.collective_compute(
                "AllGather",
                mybir.AluOpType.bypass,
                replica_groups=[list(range(C))],
                ins=[stats_in[:]],
                outs=[stats_out[:]],
            )
            st_all = cpool.tile([C, 2 * OUT], F32)
            nc.sync.dma_start(st_all[:], stats_out[:])
            st_red = cpool.tile([C, 2 * OUT], F32)
            nc.gpsimd.partition_all_reduce(
                st_red[:], st_all[:], C, bass_isa.ReduceOp.add
            )
            muex = cpool.tile([1, 2 * OUT], F32)
            var = cpool.tile([1, OUT], F32)
            srow = cpool.tile([1, OUT], F32)
            trow = cpool.tile([1, OUT], F32)
            inv_n = 1.0 / float(N)
            nc.scalar.activation(
                muex[:],
                st_red[0:1, :],
                mybir.ActivationFunctionType.Copy,
                scale=inv_n,
            )
            mu = muex[0:1, 0:OUT]
            ex2 = muex[0:1, OUT : 2 * OUT]
            nc.scalar.activation(var[:], mu, mybir.ActivationFunctionType.Square)
            nc.vector.tensor_sub(var[:], ex2, var[:])
            # var <- rsqrt(var + eps) (ACT Rsqrt is banned for accuracy)
            nc.scalar.activation(
                var[:],
                var[:],
                mybir.ActivationFunctionType.Copy,
                bias=float(cfg["EPS"]),
            )
            nc.vector.reciprocal(var[:], var[:])
            nc.scalar.activation(var[:], var[:], mybir.ActivationFunctionType.Sqrt)
            nc.vector.tensor_mul(srow[:], gm_t[:], var[:])
            nc.vector.tensor_mul(trow[:], mu, srow[:])
            nc.vector.tensor_sub(trow[:], bb_t[:], trow[:])

            S_t = cpool.tile([128, OUT], BF16)
            T_t = cpool.tile([128, OUT], BF16)
            srow_b = cpool.tile([1, OUT], BF16)
            trow_b = cpool.tile([1, OUT], BF16)
            nc.scalar.activation(
                srow_b[:], srow[:], mybir.ActivationFunctionType.Copy
            )
            nc.scalar.activation(
                trow_b[:], trow[:], mybir.ActivationFunctionType.Copy
            )
            nc.gpsimd.partition_broadcast(S_t[:], srow_b[:])
            nc.gpsimd.partition_broadcast(T_t[:], trow_b[:])

            # ---- stage G: y = relu_h * S + T (in place, bf16), chunked
            # writes so the output DMA overlaps the affine math. DRAM row
            # (g//2)*256 + p*2 + (g%2) pairs two adjacent groups of one
            # partition into 512B descriptors (slot->node map unpermutes). ----
            NGp = NG - (NG % 2)  # paired groups; odd tail written separately
            ypad_view = ypad_d[:][0 : NGp * GRP, :].rearrange(
                "(q p k) f -> p q k f", p=128, k=2
            )
            agg_pv = agg_t[:, 0:NGp, :].rearrange("p (q k) f -> p q k f", k=2)
            YC = 8
            ychunks = [(0, min(4, NG))] + [
                (g0, min(YC, NG - g0)) for g0 in range(4, NG, YC)
            ]
            for g0, gw in ychunks:
                S_bc = (
                    S_t[:]
                    .rearrange("p (o f) -> p o f", o=1)
                    .to_broadcast((128, gw, OUT))
                )
                T_bc = (
                    T_t[:]
                    .rearrange("p (o f) -> p o f", o=1)
                    .to_broadcast((128, gw, OUT))
                )
                nc.vector.tensor_tensor(
                    agg_t[:, g0 : g0 + gw, :],
                    agg_t[:, g0 : g0 + gw, :],
                    S_bc,
                    op=mybir.AluOpType.mult,
                )
                nc.vector.tensor_tensor(
                    agg_t[:, g0 : g0 + gw, :],
                    agg_t[:, g0 : g0 + gw, :],
                    T_bc,
                    op=mybir.AluOpType.add,
                )
                gp = min(g0 + gw, NGp)
                if gp > g0:
                    nc.scalar.dma_start(
                        ypad_view[:, g0 // 2 : gp // 2, :, :],
                        agg_pv[:, g0 // 2 : gp // 2, :, :],
                    )
                if g0 + gw > NGp:  # odd tail group, plain (g p) layout
                    nc.scalar.dma_start(
                        ypad_d[:]
                        .rearrange("(g p) f -> p g f", p=128)[
                            :, NGp : NG, :
                        ],
                        agg_t[:, NGp:NG, :],
                    )

    nc.compile()
    return nc


def _host_inputs(
    cfg, meta, x, W, b, gamma, beta, gidx_cores, doffA_cores, doffB_cores, degw_cores
):
    import ml_dtypes

    N, IN, OUT, C = cfg["N"], cfg["IN"], cfg["OUT"], cfg["NCORES"]
    GRP = cfg["GRP"]
    NPC, NG = meta["NPC"], meta["NG"]
    NBANKS, TROWS = meta["NBANKS"], meta["TROWS"]
    bank_nodes_cores = meta["bank_nodes_cores"]
    XK = _ceil_div(IN, 128)
    NPG = TROWS // 256
    XCOLS = NBANKS * TROWS

    # column c of xt{j}: b = c//TROWS, r = c%TROWS; pg = r//256, rem = r%256,
    # parity = rem//128, p = rem%128 -> table row pg*256 + 2p + parity;
    # table row r of bank b holds node bank_nodes[k][b][r] (per-core compact)
    c = np.arange(XCOLS, dtype=np.int64)
    bnk = c // TROWS
    r = c % TROWS
    pg = r // 256
    rem = r % 256
    parity = rem // 128
    p = rem % 128
    localrow = pg * 256 + 2 * p + parity

    xT = np.ascontiguousarray(np.asarray(x, np.float32).T)  # [IN, N]
    Wn = np.asarray(W, np.float32)
    xt_cores = []
    for k in range(C):
        bank_nodes = bank_nodes_cores[k]
        node_safe = np.zeros(XCOLS, np.int64)
        valid = np.zeros(XCOLS, np.float32)
        for bb_ in range(NBANKS):
            nrows = bank_nodes[bb_].shape[0]
            m = (bnk == bb_) & (localrow < nrows)
            node_safe[m] = bank_nodes[bb_][localrow[m]]
            valid[m] = 1.0
        xt_list = []
        for j in range(XK):
            sl = xT[j * 128 : (j + 1) * 128, :]  # [128, N]
            arr = sl[:, node_safe] * valid[None, :]
            xt_list.append(np.ascontiguousarray(arr).astype(ml_dtypes.bfloat16))
        xt_cores.append(xt_list)
    wt_list = [
        np.ascontiguousarray(Wn[j * 128 : (j + 1) * 128, :]).astype(
            ml_dtypes.bfloat16
        )
        for j in range(XK)
    ]

    iota = np.tile(
        np.arange(GRP, dtype=np.float32)[None, :], (128, 1)
    ).astype(ml_dtypes.bfloat16)
    bt = np.tile(np.asarray(b, np.float32)[None, :], (128, 1))
    onesr = np.ones((1, 128), np.float32)
    gm = np.asarray(gamma, np.float32)[None, :]
    bb = np.asarray(beta, np.float32)[None, :]
    node_of_slot = meta["node_of_slot"]
    deg_in = meta["deg_in"]

    in_maps = []
    for k in range(C):
        slots = node_of_slot[k]  # [NG*GRP], -1 = empty
        occ = slots >= 0
        degi_k = np.where(occ, deg_in[np.where(occ, slots, 0)], 1.0).astype(
            np.float32
        )
        # [128, NG] tile-major: entry (p, g) = value at slot g*GRP + p
        degi_t = np.ascontiguousarray(degi_k.reshape(NG, GRP).T)
        mask_t = np.ascontiguousarray(
            occ.astype(np.float32).reshape(NG, GRP).T
        ).astype(ml_dtypes.bfloat16)
        im = {
            "gidx": gidx_cores[k],
            "dofA": doffA_cores[k],
            "dofB": doffB_cores[k],
            "degw": degw_cores[k],
            "degi": degi_t,
            "maskg": mask_t,
            "bt": bt,
            "iota": iota,
            "gm": gm,
            "bb": bb,
            "onesr": onesr,
        }
        for j in range(XK):
            im[f"xt{j}"] = xt_cores[k][j]
            im[f"wt{j}"] = wt_list[j]
        in_maps.append(im)
    return in_maps


def kernel(x, src, dst, W, b, gamma, beta):
    global LAST_RESULTS, LAST_NC, LAST_RUN_S
    cfg = CFG
    N, E, IN, OUT, C = cfg["N"], cfg["E"], cfg["IN"], cfg["OUT"], cfg["NCORES"]
    assert x.shape == (N, IN) and W.shape == (IN, OUT)
    assert src.shape == (E,) and dst.shape == (E,)

    meta, gidx_cores, doffA_cores, doffB_cores, degw_cores = _preprocess(
        cfg, src, dst
    )
    NPC = meta["NPC"]

    nc = _build_nc(cfg, meta)
    in_maps = _host_inputs(
        cfg,
        meta,
        x,
        W,
        b,
        gamma,
        beta,
        gidx_cores,
        doffA_cores,
        doffB_cores,
        degw_cores,
    )

    if cfg.get("SIM"):
        from concourse.bass_interp import MultiCoreSim

        sim = MultiCoreSim(nc, num_cores=C)
        for k, core_sim in sim.cores.items():
            for name, val in in_maps[k].items():
                core_sim.tensor(name)[:] = val
        sim.simulate()
        y = np.empty((N, OUT), np.float32)
        node_of_slot = meta["node_of_slot"]
        rows = _row_of_slot(meta["NG"], cfg["GRP"])
        for k in range(C):
            slots = node_of_slot[k]
            occ = slots >= 0
            yp = np.asarray(sim.cores[k].tensor("ypad"), np.float32)
            y[slots[occ]] = yp[rows[occ]]
        return y

    LAST_NC = nc
    import time as _time

    _t0 = _time.time()
    res = bass_utils.run_bass_kernel_spmd(
        nc,
        in_maps,
        core_ids=list(range(C)),
        trace=cfg.get("TRACE", False),
    )
    LAST_RUN_S = _time.time() - _t0
    LAST_RESULTS = res

    y = np.empty((N, OUT), np.float32)
    node_of_slot = meta["node_of_slot"]
    rows = _row_of_slot(meta["NG"], cfg["GRP"])
    for k in range(C):
        slots = node_of_slot[k]
        occ = slots >= 0
        yp = np.asarray(res.results[k]["ypad"], np.float32)
        y[slots[occ]] = yp[rows[occ]]
    return y

